# revision 34
# baseline (speedup 1.0000x reference)
"""Sharded sparse (windowed) attention for TRN2 — 8 NeuronCores, head-parallel.

Reference computation (B=4, N=197, C=2048, H=32 heads, hd=64, window=8):
    qkv = x @ qkv_w.T -> split q,k,v per head
    attn = softmax(mask_weight * (q@k.T) * hd^-0.5  with off-band -inf)
    out  = (attn @ v) per head, concat heads, @ proj_w.T + proj_b

Sharding: 4 heads per core (tensor parallel). Each core computes its heads'
qkv projection, windowed attention, and a partial of the output projection
(contraction over its 256 head-dims). Host sums the 8 partials + bias.

On-device layout is fully transposed (feature dim on partitions, tokens on
the free axis) so no transposes are ever needed:
    xT (2048, 788)  qkT (512, 788)  v (tokens, 256)  E=(j,i)  outT (2048, 788)

The banded score matrix is computed as two blocks packed side-by-side in one
PSUM tile: cols [0:136) = rows j<128 x tokens i<136; cols [136:214) =
rows j in [128,197) x tokens i in [120,198).
"""

import numpy as np

B = 4
N = 197
C = 2048
H = 32
HD = 64
WIN = 8
NCORES = 8
HPC = H // NCORES          # heads per core
CPC = HPC * HD             # head-dims per core (256)
T = B * N                  # 788 tokens
TP = T + 2                 # padded qkT width (block-1 rhs reads col 788)
KC = C // 128              # 16 contraction chunks
SCALE = HD ** -0.5
NEG = -200.0               # additive off-band mask (exp underflows to 0)

# banded blocks of ST[j, i]: (j0, jh, i0, iw, packed column offset)
# both blocks are 128 j-rows (they overlap in j; the mask zeroes block 1's
# rows j<128 so overlapped pairs count once). No partial-partition corners.
BLOCKS = [(0, 128, 0, 136, 0), (69, 128, 62, 136, 136)]
SW = 272                   # packed score-tile width (136 + 136)
NP = 198                   # padded zt width
TOKCH = [(0, 128), (69, 128)]          # per-batch token chunks (v rows)
QKCH_F32R = [(0, 394), (394, 394)]
QKCH_BF16 = [(0, 394), (394, 394)]
PRHALF = [(0, 394), (394, 394)]        # proj column halves (per batch pair)

# f32r (fp32 rounded to 11 mantissa bits, full PE rate) keeps the end-to-end
# relative error at ~3e-4; the bf16/bf16 config runs ~18% faster (102us vs
# 120us) at ~4.8e-3 error if a looser tolerance is acceptable.
DT_BIG = "f32r"            # qkv + proj matmul operand dtype: f32r | bf16
DT_ATT = "f32r"            # attention matmul operand dtype:  f32r | bf16

_compiled = {}


def _dt(mybir, name):
    return {"f32r": mybir.dt.float32r, "bf16": mybir.dt.bfloat16}[name]


def _build_program(dt_big, dt_att):
    import concourse.mybir as mybir
    import concourse.tile as tile
    from concourse import bacc

    F32 = mybir.dt.float32
    DTB = _dt(mybir, dt_big)
    DTA = _dt(mybir, dt_att)

    QKCH = QKCH_BF16 if dt_big == "bf16" else QKCH_F32R

    nc = bacc.Bacc("TRN2", target_bir_lowering=False, debug=False)

    # xw = [xT | wqkT | wvT] packed on the 2048-row contraction axis
    XWW = T + 2 * CPC + CPC
    xw = nc.dram_tensor("xw", [C, XWW], DTB, kind="ExternalInput")
    pw = nc.dram_tensor("pw", [CPC, C], DTB, kind="ExternalInput")
    maskA = nc.dram_tensor("maskA", [128, SW], F32, kind="ExternalInput")
    maskM = nc.dram_tensor("maskM", [128, SW], F32, kind="ExternalInput")
    outT = nc.dram_tensor("outT", [C, T], F32, kind="ExternalOutput")

    with tile.TileContext(nc) as tc:
        with (
            tc.tile_pool(name="persist", bufs=1) as per,
            tc.tile_pool(name="work", bufs=6) as wk,
            tc.tile_pool(name="ps", bufs=2, space="PSUM") as pp,
        ):
            # ---- small constants first (mask tiles double as PE warm-up fodder) ----
            mA = per.tile([128, SW], F32, tag="mA")
            nc.sync.dma_start(out=mA, in_=maskA[:, :])
            mM = per.tile([128, SW], F32, tag="mM")
            nc.sync.dma_start(out=mM, in_=maskM[:, :])
            pw_t = []
            for k2 in range(2):
                t = per.tile([128, C], DTB, tag=f"pw{k2}")
                nc.sync.dma_start(out=t, in_=pw[k2 * 128:(k2 + 1) * 128, :])
                pw_t.append(t)
            onecol = per.tile([128, 1], F32, tag="onecol")
            nc.vector.memset(onecol, 1.0)
            zpad = per.tile([128, TP - T], F32, tag="zpad")
            nc.vector.memset(zpad, 0.0)

            # warm-up matmuls on memset data (no DMA dependency): keeps the
            # PE busy while the first x/w chunks stream in, so HAM
            # un-throttles the clock early.
            wusrc = per.tile([128, 256], F32, tag="wusrc")
            nc.vector.memset(wusrc, 1.0)
            wu = pp.tile([128, 256], F32, tag="mm", name="wu")
            for _ in range(10):
                nc.tensor.matmul(out=wu, lhsT=wusrc[:, 0:128], rhs=wusrc,
                                 start=True, stop=True)

            def wu_fill(n, tag):
                w = pp.tile([128, 256], F32, tag="st", name=f"wu_{tag}", bufs=4)
                for _ in range(n):
                    nc.tensor.matmul(out=w, lhsT=wusrc[:, 0:128], rhs=wusrc,
                                     start=True, stop=True)

            # ---- input loads: one DMA per contraction chunk ----
            xwt = []
            for kc in range(KC):
                t = per.tile([128, XWW], DTB, tag=f"xw{kc}")
                ks = slice(kc * 128, (kc + 1) * 128)
                if kc < 2:
                    nc.sync.dma_start(out=t[:, T:T + CPC], in_=xw[ks, T:T + CPC])
                    nc.sync.dma_start(out=t[:, T + CPC:XWW], in_=xw[ks, T + CPC:XWW])
                    nc.sync.dma_start(out=t[:, 0:394], in_=xw[ks, 0:394])
                    nc.sync.dma_start(out=t[:, 394:T], in_=xw[ks, 394:T])
                else:
                    nc.sync.dma_start(out=t[:, T:XWW], in_=xw[ks, T:XWW])
                    nc.sync.dma_start(out=t[:, 0:T], in_=xw[ks, 0:T])
                xwt.append(t)
            xt = [t[:, 0:T] for t in xwt]
            wqk_t = [t[:, T:T + 2 * CPC] for t in xwt]
            wv_t = [t[:, T + 2 * CPC:XWW] for t in xwt]

            # ---- phase 1: q,k projection (weight-stationary) ----
            # qkT[mc] rows: mc 0,1 = q (heads 0,1 / 2,3); mc 2,3 = k
            # mc alternates innermost so consecutive matmuls load different
            # weights (LDWEIGHTS overlaps via the background weight buffer).
            qkT = []
            for mc4 in range(4):
                t = per.tile([128, TP], DTA, tag=f"qkT{mc4}", name=f"qkT{mc4}")
                nc.vector.tensor_copy(out=t[:, T:TP], in_=zpad)
                qkT.append(t)
            gtag = {0: "mm", 1: "st", 2: "st", 3: "zt"}
            qps = {}
            for mc in range(4):
                for ni, (n0, nw) in enumerate(QKCH):
                    qps[(mc, ni)] = pp.tile(
                        [128, nw], F32, tag=gtag[mc], name=f"qkps{mc}_{ni}",
                        bufs=4 if gtag[mc] == "st" else 2)
            for kc in range(KC):
                for ni, (n0, nw) in enumerate(QKCH):
                    for mc in range(4):
                        nc.tensor.matmul(
                            out=qps[(mc, ni)],
                            lhsT=wqk_t[kc][:, mc * 128:(mc + 1) * 128],
                            rhs=xt[kc][:, n0:n0 + nw],
                            start=(kc == 0), stop=(kc == KC - 1),
                        )
            for mc in range(4):
                for ni, (n0, nw) in enumerate(QKCH):
                    if (mc + ni) % 2 == 0:
                        nc.scalar.copy(out=qkT[mc][:, n0:n0 + nw],
                                       in_=qps[(mc, ni)])
                    else:
                        nc.vector.tensor_copy(out=qkT[mc][:, n0:n0 + nw],
                                              in_=qps[(mc, ni)])

            # ---- phase 2: v projection (x-stationary: v in (tokens, dims)) ----
            vone = {}  # (b, jc) -> [th, 4, 65] tile (per-head v cols + ones col)

            def emit_v(b, jc):
                t0, th = TOKCH[jc]
                vps = pp.tile([th, CPC], F32, tag="mm", name=f"vps{b}_{jc}")
                for kc in range(KC):
                    nc.tensor.matmul(
                        out=vps,
                        lhsT=xt[kc][:, b * N + t0: b * N + t0 + th],
                        rhs=wv_t[kc],
                        start=(kc == 0), stop=(kc == KC - 1),
                    )
                vt = per.tile([th, HPC, HD + 1], DTA, tag=f"vone{b}_{jc}",
                              name=f"vone{b}_{jc}")
                nc.vector.tensor_copy(
                    out=vt[:, :, 0:HD],
                    in_=vps.rearrange("t (h d) -> t h d", h=HPC))
                nc.vector.tensor_copy(
                    out=vt[:, :, HD],
                    in_=onecol[:th, 0:1].to_broadcast((th, HPC)))
                vone[(b, jc)] = vt

            for b in (0, 1):
                for jc in range(2):
                    emit_v(b, jc)

            # ---- phase 3 + 4: windowed attention, proj interleaved ----
            # per (b,h): ST packed [128, 214]; logits = (ST + A) * M
            # E = exp(logits); zT' = [v|1]^T @ E  (row HD = softmax denom)
            # After batches (0,1) / (2,3): project the finished column half.
            YT = [per.tile([128, T], DTB, tag=f"YT{k2}", name=f"YT{k2}")
                  for k2 in range(2)]

            def attn_s(b, h):
                """score matmuls for (b,h) -> st psum tile"""
                po = 64 * (h % 2)
                qTh = qkT[h // 2][po:po + 64, :]
                kTh = qkT[2 + h // 2][po:po + 64, :]
                st = pp.tile([128, SW], F32, tag="st", name=f"st{b}_{h}", bufs=4)
                for (j0, jh, i0, iw, c0) in BLOCKS:
                    nc.tensor.matmul(
                        out=st[0:jh, c0:c0 + iw],
                        lhsT=kTh[:, b * N + j0: b * N + j0 + jh],
                        rhs=qTh[:, b * N + i0: b * N + i0 + iw],
                        start=True, stop=True,
                    )
                return st

            def attn_rest(b, h, st):
                """mask, exp, AV, decoupled normalize for (b,h)"""
                po = 64 * (h % 2)
                sa = wk.tile([128, SW], F32, tag="sa")
                nc.vector.tensor_tensor(sa, st, mA, mybir.AluOpType.add)
                sm = wk.tile([128, SW], F32, tag="sm")
                nc.vector.tensor_tensor(sm, sa, mM, mybir.AluOpType.mult)
                e = wk.tile([128, SW], DTA, tag="e")
                nc.scalar.activation(
                    out=e, in_=sm, func=mybir.ActivationFunctionType.Exp)
                zt = pp.tile([HD + 1, NP], F32, tag="zt", name=f"zt{b}_{h}")
                for blk, (j0, jh, i0, iw, c0) in enumerate(BLOCKS):
                    nc.tensor.matmul(
                        out=zt[:, i0:i0 + iw],
                        lhsT=vone[(b, blk)][0:jh, h, :],
                        rhs=e[0:jh, c0:c0 + iw],
                        start=(blk == 0), stop=(blk == len(BLOCKS) - 1),
                    )
                # copy z+den out of PSUM quickly to free the bank, then the
                # normalization tail runs off the critical path.
                ztmp = wk.tile([HD, N], F32, tag="ztmp")
                nc.scalar.copy(out=ztmp, in_=zt[0:HD, 0:N])
                dent = wk.tile([1, N], F32, tag="dent")
                nc.scalar.copy(out=dent, in_=zt[HD:HD + 1, 0:N])
                rrow = wk.tile([1, N], F32, tag="rrow")
                nc.vector.reciprocal_approx_fast(out=rrow, in_=dent)
                rb = wk.tile([64, N], F32, tag="rb")
                nc.gpsimd.partition_broadcast(rb, rrow)
                nc.vector.tensor_tensor(
                    YT[h // 2][po:po + 64, b * N:(b + 1) * N],
                    ztmp, rb, mybir.AluOpType.mult)

            def proj_cols(n0, nw, label, mcs):
                for mc in mcs:
                    pps = pp.tile([128, nw], F32, tag="mm", name=f"pj{label}_{mc}")
                    for k2 in range(2):
                        nc.tensor.matmul(
                            out=pps,
                            lhsT=pw_t[k2][:, mc * 128:(mc + 1) * 128],
                            rhs=YT[k2][:, n0:n0 + nw],
                            start=(k2 == 0), stop=(k2 == 1),
                        )
                    ob = wk.tile([128, nw], F32, tag="ob", name=f"ob{label}_{mc}")
                    nc.scalar.copy(out=ob, in_=pps)
                    nc.sync.dma_start(
                        out=outT[mc * 128:(mc + 1) * 128, n0:n0 + nw], in_=ob)

            # software-pipeline: emit S matmuls one (b,h) ahead so the PE has
            # independent work while the previous chain's DVE/ACT stages run.
            # Finished batches' proj matmuls are spread between the next
            # batch's head chains (4 M-chunks per head slot) so the PE never
            # idles long enough for HAM to re-throttle the clock.
            MCG = KC // HPC  # proj M-chunks emitted per head slot
            bhs = [(b, h) for b in range(B) for h in range(HPC)]
            vfill = [(2, 0), (2, 1), (3, 0), (3, 1)]  # v-groups used as filler
            sts = {}
            sts[bhs[0]] = attn_s(*bhs[0])
            sts[bhs[1]] = attn_s(*bhs[1])
            for idx, (b, h) in enumerate(bhs):
                if idx % 2 == 0:
                    for j in (idx + 2, idx + 3):
                        if j < len(bhs):
                            sts[bhs[j]] = attn_s(*bhs[j])
                attn_rest(b, h, sts.pop((b, h)))
                if h % 2 == 1 and vfill:
                    emit_v(*vfill.pop(0))
                if b == 2:
                    proj_cols(0, 2 * N, "p0", range(h * MCG, (h + 1) * MCG))
            proj_cols(2 * N, 2 * N, "p1", range(KC))

    nc.compile()
    return nc


def _host_masks():
    i = np.arange(N)[:, None]
    j = np.arange(N)[None, :]
    d = np.abs(i - j).astype(np.float32)
    in_win = (j >= i - WIN) & (j < i + WIN)
    m = np.where(in_win, (WIN - d / 2.0) / WIN, 0.0).astype(np.float32)
    # transposed (j on rows): logits[j,i] = (ST[j,i] + A[j,i]) * M[j,i]
    multT = np.where(in_win, m * SCALE, 1.0).astype(np.float32).T
    addT = np.where(in_win, 0.0, NEG).astype(np.float32).T
    # pack the two banded blocks side by side into [128, SW] tiles
    mult = np.ones((128, SW), dtype=np.float32)
    addm = np.full((128, SW), NEG, dtype=np.float32)
    for blk, (j0, jh, i0, iw, c0) in enumerate(BLOCKS):
        iw_r = min(iw, N - i0)  # data columns (rest stays pad)
        mult[0:jh, c0:c0 + iw_r] = multT[j0:j0 + jh, i0:i0 + iw_r]
        addm[0:jh, c0:c0 + iw_r] = addT[j0:j0 + jh, i0:i0 + iw_r]
        if blk == 1 and j0 < 128:
            # rows j<128 belong to block 0 — kill them here
            kill = 128 - j0
            mult[0:kill, c0:c0 + iw] = 1.0
            addm[0:kill, c0:c0 + iw] = NEG
    return addm, mult


def _np_dt(name):
    if name == "bf16":
        import ml_dtypes
        return ml_dtypes.bfloat16
    return np.float32


def _make_in_maps(x, qkv_w, proj_w):
    npb = _np_dt(DT_BIG)
    xT = x.reshape(T, C).T
    addm, mult = _host_masks()
    in_maps = []
    for d in range(NCORES):
        r = slice(d * CPC, (d + 1) * CPC)
        wqk_d = np.concatenate(
            [qkv_w[r, :], qkv_w[C + d * CPC: C + (d + 1) * CPC, :]], axis=0).T
        wv_d = qkv_w[2 * C + d * CPC: 2 * C + (d + 1) * CPC, :].T
        xw_d = np.ascontiguousarray(
            np.concatenate([xT, wqk_d, wv_d], axis=1)).astype(npb)
        pw_d = np.ascontiguousarray(proj_w[:, r].T).astype(npb)
        in_maps.append({"xw": xw_d, "pw": pw_d, "maskA": addm, "maskM": mult})
    return in_maps


def kernel(x, qkv_w, proj_w, proj_b):
    from concourse.bass_utils import run_bass_kernel_spmd

    key = (DT_BIG, DT_ATT)
    if key not in _compiled:
        _compiled[key] = _build_program(*key)
    nc = _compiled[key]

    x = np.asarray(x, dtype=np.float32)
    qkv_w = np.asarray(qkv_w, dtype=np.float32)
    proj_w = np.asarray(proj_w, dtype=np.float32)
    proj_b = np.asarray(proj_b, dtype=np.float32)

    in_maps = _make_in_maps(x, qkv_w, proj_w)
    res = run_bass_kernel_spmd(nc, in_maps, core_ids=list(range(NCORES)))
    acc = np.zeros((C, T), dtype=np.float32)
    for r in res.results:
        acc += r["outT"]
    out = acc.T + proj_b[None, :]
    return np.ascontiguousarray(out).reshape(B, N, C)
